# revision 3
# baseline (speedup 1.0000x reference)
"""Trainium2 Bass kernel for nn_AutoregressiveDecoder.

Reference computation (B=2048, T=1024, D=32, S=2):
    s_{t+1} = s_t @ Ws.T + z_t @ Wz.T        (Ws = W[:, :2], Wz = W[:, 2:])
    out[:, t] = s_t,  s_0 = init_states      -> (B, T, S) fp32

Strategy: data-parallel over 8 cores (256 batch rows each). The sequential
scan is re-expressed as 16 chunks of C=64 steps; within a chunk all 64
outputs are produced by ONE PE accumulation group against a host-precomputed
block-Toeplitz matrix Q[(tl,d),(j,s)] = (Wz^T M^{j-1-tl})[d,s] (M = Ws^T),
plus a carry-in term via R[(s'),(j,s)] = M^j. The inter-chunk carry is a
16-step chain of tiny matmuls. Matmuls run in float32r (11-bit mantissa,
1 cycle/row at N=256); the carry chain uses hi/lo splitting so its error
stays at fp32 level. z is transposed on-chip with PE transposes (batch must
leave the partition dim for the contraction); output stays in (t, b) layout
and is transposed on the host.

Output rows use REVERSED j order (row = (C-1-j)*S + s) so that the last
state of a chunk lands on partitions 0:2 (partition bases must be 32-aligned
on this hardware).
"""

import numpy as np

B, T, D, S = 2048, 1024, 32, 2
C = 64                  # time steps per chunk
NCORES = 8
BL = B // NCORES        # 256 batch rows per core
KT = C * D // 128       # 16 K-tiles of 128 per chunk


# ---------------------------------------------------------------------------
# host-side helpers
# ---------------------------------------------------------------------------

def _rne11(v):
    """Round fp32 to 11 mantissa bits, round-to-nearest-even — bit-exact model
    of the TRN2 float32r rounding (verified on hardware)."""
    v = np.ascontiguousarray(v, np.float32)
    u = v.view(np.uint32)
    low = u & np.uint32(0xFFF)
    keep = u & np.uint32(0xFFFFF000)
    lsb = (u >> np.uint32(12)) & np.uint32(1)
    up = (low > 0x800) | ((low == 0x800) & (lsb == 1))
    r = keep + (up.astype(np.uint32) << np.uint32(12))
    return r.view(np.float32)


def _host_constants(W):
    """Build Q/R/M/u-last operand matrices from W (fp64 powers, fp32 out).

    Output column index m = (C-1-j)*S + s  (reversed j)."""
    W64 = W.astype(np.float64)
    M = W64[:, :S].T            # (2, 2)
    WzT = W64[:, S:].T          # (32, 2)

    Mpow = [np.eye(S)]
    for _ in range(C + 1):
        Mpow.append(Mpow[-1] @ M)

    def col(j):
        return (C - 1 - j) * S

    Q = np.zeros((C * D, C * S), np.float64)
    for j in range(1, C):
        for tl in range(j):
            Q[tl * D:(tl + 1) * D, col(j):col(j) + S] = WzT @ Mpow[j - 1 - tl]
    R = np.zeros((S, C * S), np.float64)
    for j in range(C):
        R[:, col(j):col(j) + S] = Mpow[j]

    Rf = R.astype(np.float32)
    R_hi = _rne11(Rf)
    R_lo = _rne11(Rf - R_hi)
    Mf = M.astype(np.float32)           # lhsT layout: lhsT[k=s', m=s] = M[s', s]
    M_hi = _rne11(Mf)
    M_lo = _rne11(Mf - M_hi)

    # M operands padded to M=128 output columns (only cols 0:2 meaningful)
    mhi2 = np.zeros((2, 128), np.float32)
    mhi2[:, 0:S] = M_hi
    mlo2 = np.zeros((2, 128), np.float32)
    mlo2[:, 0:S] = M_lo

    # u_last operand: full K=128 tile, only rows 96:128 (t-local 63) nonzero
    ulast = np.zeros((128, 128), np.float32)
    ulast[96:128, 0:S] = WzT.astype(np.float32)

    qmat = np.ascontiguousarray(
        Q.astype(np.float32).reshape(KT, 128, C * S))        # (16, 128, 128)

    return {
        "qmat": qmat,
        "rhi2": np.ascontiguousarray(R_hi),                   # (2, 128)
        "rlo2": np.ascontiguousarray(R_lo),                   # (2, 128)
        "mhi2": mhi2,
        "mlo2": mlo2,
        "ulast": ulast,
        "ident": np.eye(128, dtype=np.float32),
    }


# ---------------------------------------------------------------------------
# workarounds for this container's walrus (max 1 sem-wait per instruction)
# ---------------------------------------------------------------------------

def _install_patches():
    import concourse.tile as tile
    import concourse.mybir as mybir
    from bass_rust import ScopedClock

    if getattr(tile.TileContext, "_ard_patched", False):
        return

    def _drain_and_barrier(self, tick_clock, wait_clock):
        nc = self.nc
        probe = nc.sync.nop(nofuse=True, hint="tail_wait_spread")
        wait_clock.add_sem_waits(
            probe.ins, ScopedClock({None: tick_clock.global_clock})
        )
        si = probe.ins.sync_info
        waits = list(si.on_wait) if si is not None else []
        updates = list(si.on_update) if si is not None else []
        if len(waits) > 1:
            probe.ins.sync_info = mybir.SyncInfo(on_wait=waits[:1], on_update=updates)
            for w in waits[1:]:
                n2 = nc.sync.nop(nofuse=True, hint="tail_wait_spread")
                n2.ins.sync_info = mybir.SyncInfo(on_wait=[w], on_update=[])
        nc.sync.drain()
        nc.all_engine_barrier()
        assert self.sems is not None
        popped = nc._tile_sem_poison_stack.pop()
        assert popped is self._sem_poison
        nc.clear_and_free_semaphores(list(self.sems.allocated().values()))
        nc.all_engine_barrier()

    tile.TileContext._drain_and_barrier = _drain_and_barrier
    tile.TileContext._ard_patched = True


def _spread_waits(nc):
    """Move excess sem-waits (>1) onto same-engine NoOps inserted just before
    the owning instruction (engines are in-order, so semantics hold)."""
    import concourse.mybir as mybir

    ctr = 0
    for f in nc.m.functions:
        for b in f.blocks:
            out = []
            changed = False
            for inst in b.instructions:
                si = inst.sync_info
                waits = list(si.on_wait) if si is not None else []
                if len(waits) > 1 and inst.engine != mybir.EngineType.Unassigned:
                    changed = True
                    for w in waits[:-1]:
                        ctr += 1
                        out.append(
                            mybir.InstNoOp(
                                name=f"waitspread-{ctr}",
                                sync_info=mybir.SyncInfo(on_wait=[w], on_update=[]),
                                bass_nofuse=True,
                                engine=inst.engine,
                            )
                        )
                    inst.sync_info = mybir.SyncInfo(
                        on_wait=waits[-1:], on_update=list(si.on_update)
                    )
                out.append(inst)
            if changed:
                b.instructions = out
    return ctr


# ---------------------------------------------------------------------------
# device program
# ---------------------------------------------------------------------------

def _build_nc(nch):
    import concourse.bass as bass
    import concourse.tile as tile
    import concourse.mybir as mybir

    _install_patches()
    f32 = mybir.dt.float32
    f32r = mybir.dt.float32r
    PSUM = bass.MemorySpace.PSUM
    Tl = nch * C

    nc = bass.Bass(trn_type="TRN2", target_bir_lowering=False, debug=False)
    zin = nc.dram_tensor("zin", [BL, Tl, D], f32r, kind="ExternalInput")
    qmat = nc.dram_tensor("qmat", [KT, 128, C * S], f32r, kind="ExternalInput")
    rhi2 = nc.dram_tensor("rhi2", [2, C * S], f32r, kind="ExternalInput")
    rlo2 = nc.dram_tensor("rlo2", [2, C * S], f32r, kind="ExternalInput")
    mhi2 = nc.dram_tensor("mhi2", [2, 128], f32r, kind="ExternalInput")
    mlo2 = nc.dram_tensor("mlo2", [2, 128], f32r, kind="ExternalInput")
    ulast = nc.dram_tensor("ulast", [128, 128], f32r, kind="ExternalInput")
    ident = nc.dram_tensor("ident", [128, 128], f32r, kind="ExternalInput")
    c0hi = nc.dram_tensor("c0hi", [2, BL], f32r, kind="ExternalInput")
    c0lo = nc.dram_tensor("c0lo", [2, BL], f32r, kind="ExternalInput")
    out = nc.dram_tensor("out", [128, nch * BL], f32, kind="ExternalOutput")

    with tile.TileContext(nc) as tc:
        with (
            tc.tile_pool(name="const", bufs=1) as const,
            tc.tile_pool(name="zload", bufs=6) as zload,
            tc.tile_pool(name="ztsb", bufs=16) as ztsb,
            tc.tile_pool(name="obuf", bufs=2) as obuf,
            tc.tile_pool(name="chib", bufs=3) as chib,
            tc.tile_pool(name="clob", bufs=3) as clob,
            tc.tile_pool(name="ztps", bufs=4, space=PSUM) as ztps,
            tc.tile_pool(name="outps", bufs=2, space=PSUM) as outps,
            tc.tile_pool(name="cps", bufs=2, space=PSUM) as cpsp,
        ):
            qsb = const.tile([128, KT * C * S], f32r)
            nc.sync.dma_start(
                qsb[:].rearrange("p (k m) -> p k m", k=KT),
                qmat.ap().rearrange("k p m -> p k m"),
            )
            rhisb = const.tile([2, C * S], f32r)
            nc.sync.dma_start(rhisb[:], rhi2.ap())
            rlosb = const.tile([2, C * S], f32r)
            nc.sync.dma_start(rlosb[:], rlo2.ap())
            mhisb = const.tile([2, 128], f32r)
            nc.sync.dma_start(mhisb[:], mhi2.ap())
            mlosb = const.tile([2, 128], f32r)
            nc.sync.dma_start(mlosb[:], mlo2.ap())
            ulsb = const.tile([128, 128], f32r)
            nc.sync.dma_start(ulsb[:], ulast.ap())
            idsb = const.tile([128, 128], f32r)
            nc.sync.dma_start(idsb[:], ident.ap())

            chi = chib.tile([2, BL], f32r)
            nc.sync.dma_start(chi[:], c0hi.ap())
            clo = clob.tile([2, BL], f32r)
            nc.sync.dma_start(clo[:], c0lo.ap())

            ob = None
            for k in range(nch):
                # ---- load z chunk (2 x 1 MB, contiguous per partition) ----
                znat = []
                for bg in range(2):
                    zt = zload.tile([128, C * D], f32r)
                    nc.sync.dma_start(
                        zt[:],
                        zin.ap()[bg * 128:(bg + 1) * 128, k * C:(k + 1) * C, :]
                        .rearrange("p t d -> p (t d)"),
                    )
                    znat.append(zt)

                # ---- transpose to (td, b) layout; PSUM -> SBUF copies ----
                ztiles = []
                for g in range(KT // 2):
                    ztp = ztps.tile([128, 512], f32r)
                    for h in range(2):
                        kt = 2 * g + h
                        for bg in range(2):
                            off = h * 256 + bg * 128
                            nc.tensor.transpose(
                                ztp[:, off:off + 128],
                                znat[bg][:, kt * 128:(kt + 1) * 128],
                                idsb[:],
                            )
                    zsb = ztsb.tile([128, 512], f32r)
                    if g < 3:
                        nc.vector.tensor_copy(zsb[:], ztp[:])
                    else:
                        nc.scalar.copy(zsb[:], ztp[:])
                    ztiles.append(zsb)

                # ---- conv + carry-add: one PSUM accumulation group ----
                pout = outps.tile([128, BL], f32)
                for kt in range(KT):
                    nc.tensor.matmul(
                        pout[:],
                        qsb[:, kt * 128:(kt + 1) * 128],
                        ztiles[kt // 2][:, (kt % 2) * 256:(kt % 2) * 256 + 256],
                        start=(kt == 0),
                        stop=False,
                    )
                nc.tensor.matmul(pout[:], rhisb[:], chi[:], start=False, stop=False)
                nc.tensor.matmul(pout[:], rhisb[:], clo[:], start=False, stop=False)
                nc.tensor.matmul(pout[:], rlosb[:], chi[:], start=False, stop=True)

                # ---- stage outputs (ACT); 4 chunks per out buffer ----
                if k % 4 == 0:
                    ob = obuf.tile([128, 4 * BL], f32)
                col = (k % 4) * BL
                nc.scalar.copy(ob[:, col:col + BL], pout[:])

                # ---- carry update (s_last lives on partitions 0:2) ----
                if k < nch - 1:
                    shi = chib.tile([2, BL], f32r)
                    nc.vector.tensor_copy(shi[:], ob[0:2, col:col + BL])
                    slo = clob.tile([2, BL], f32r)
                    nc.vector.tensor_sub(
                        slo[:], ob[0:2, col:col + BL], shi[:].bitcast(f32)
                    )
                    cp = cpsp.tile([128, BL], f32)
                    nc.tensor.matmul(cp[:], mhisb[:], shi[:], start=True, stop=False)
                    nc.tensor.matmul(cp[:], mhisb[:], slo[:], start=False, stop=False)
                    nc.tensor.matmul(cp[:], mlosb[:], shi[:], start=False, stop=False)
                    nc.tensor.matmul(cp[:], ulsb[:], ztiles[KT // 2 - 1][:, 256:512],
                                     start=False, stop=True)
                    chi = chib.tile([2, BL], f32r)
                    nc.vector.tensor_copy(chi[:], cp[0:2, :])
                    clo = clob.tile([2, BL], f32r)
                    nc.vector.tensor_sub(clo[:], cp[0:2, :], chi[:].bitcast(f32))

                if k % 4 == 3 or k == nch - 1:
                    k0 = (k // 4) * 4
                    w = (k - k0 + 1) * BL
                    nc.sync.dma_start(
                        out.ap()[:, k0 * BL:k0 * BL + w], ob[:, 0:w]
                    )

    _spread_waits(nc)
    return nc


_CACHE = {}


def _get_nc(nch):
    if nch not in _CACHE:
        _CACHE[nch] = _build_nc(nch)
    return _CACHE[nch]


# ---------------------------------------------------------------------------
# entry point
# ---------------------------------------------------------------------------

def _run(init_states, z, W, nch, core_ids, trace=False):
    from concourse.bass_utils import run_bass_kernel_spmd

    consts = _host_constants(W)
    ncores = len(core_ids)
    in_maps = []
    for i in range(ncores):
        sl = slice(i * BL, (i + 1) * BL)
        init_T = np.ascontiguousarray(init_states[sl].T, np.float32)  # (2, BL)
        hi = _rne11(init_T)
        lo = _rne11(init_T - hi)
        in_maps.append({
            "zin": np.ascontiguousarray(z[sl, :nch * C, :], np.float32),
            "qmat": consts["qmat"],
            "rhi2": consts["rhi2"],
            "rlo2": consts["rlo2"],
            "mhi2": consts["mhi2"],
            "mlo2": consts["mlo2"],
            "ulast": consts["ulast"],
            "ident": consts["ident"],
            "c0hi": hi,
            "c0lo": lo,
        })

    nc = _get_nc(nch)
    kwargs = {}
    if trace:
        kwargs = dict(trace=True, trace_cores=list(core_ids))
    res = run_bass_kernel_spmd(nc, in_maps, core_ids=list(core_ids), **kwargs)

    outs = []
    for i in range(ncores):
        o = res.results[i]["out"]                       # (128, nch*BL)
        o = o.reshape(C, S, nch, BL)                    # (rev_j, s, k, b)
        o = o[::-1]                                     # undo reversed j
        o = np.transpose(o, (3, 2, 0, 1)).reshape(BL, nch * C, S)
        outs.append(o)
    full = np.concatenate(outs, axis=0).astype(np.float32)
    return full, res


def kernel(init_states, z, W):
    full, _ = _run(init_states, z, W, T // C, list(range(NCORES)))
    return full
